# revision 1
# baseline (speedup 1.0000x reference)
"""Trainium2 Bass kernel for the topk-masking attention module.

Computation (per sample n):
    cams[k, hw] = relu(sum_c x[n, c, hw] * w[k, c])          # 1x1 conv, K=4
    thr[k]      = gama * max_hw(cams[k, :])
    dropped     = where(cams > thr, 0, cams)
    mean[hw]    = sum_k dropped[k, hw] / 4
    out[n,c,hw] = x[n,c,hw] * mean[hw]

Strategy: data-parallel over batch N=32 across 8 NeuronCores (4 samples
per core).  Per sample, x[n] ([4096, 784] f32, 12.85 MB) is loaded into
SBUF once as 4 quarter tiles [128, 8, 784] and kept resident: the 1x1
conv runs as 32 accumulating fp32 matmuls (contraction 4096 = 32x128 on
partitions) into PSUM [4, 784]; the per-channel max / threshold / mask
run on ACT+DVE; the channel-mean + broadcast to 128 partitions is a
single matmul with a constant [4, 128] lhsT of 0.25; and the final
elementwise multiply reuses the resident x tiles in-place before the
store.  Total HBM traffic is the 2x floor (read x once, write out once).
"""

import sys

for _p in ("/opt/trn_rl_repo",):
    if _p not in sys.path:
        sys.path.insert(0, _p)

import numpy as np

N_CORES = 8
NFULL = 32            # full batch
NS = NFULL // N_CORES  # samples per core
C = 4096
K = 4
HW = 28 * 28          # 784
NCHUNK = C // 128     # 32
NQ = 4                # quarter tiles per sample
CPQ = NCHUNK // NQ    # 8 chunks per quarter
HALVES = ((0, 512), (512, HW))  # PSUM-bank-aligned column split

_CACHE = {}


def build_nc(x_bufs=6, cams_bufs=2, mean_bufs=2):
    """Trace + schedule + compile the per-core Bass program."""
    from contextlib import ExitStack

    import concourse.bacc as bacc
    import concourse.tile as tile
    from concourse import mybir

    f32 = mybir.dt.float32
    nc = bacc.Bacc("TRN2", target_bir_lowering=False, debug=False,
                   num_devices=N_CORES)

    x_d = nc.dram_tensor("x", [NS, C, HW], f32, kind="ExternalInput")
    w_d = nc.dram_tensor("w", [128, NCHUNK, K], f32, kind="ExternalInput")
    gam_d = nc.dram_tensor("gam", [K, 1], f32, kind="ExternalInput")
    qlhs_d = nc.dram_tensor("qlhs", [K, 128], f32, kind="ExternalInput")
    out_d = nc.dram_tensor("out", [NS, C, HW], f32, kind="ExternalOutput")

    # [NS, C, HW] viewed as [NS, NQ, 128(part), CPQ, HW]
    x_src = x_d.ap().rearrange("n (q j p) hw -> n q p j hw", q=NQ, j=CPQ, p=128)
    out_dst = out_d.ap().rearrange("n (q j p) hw -> n q p j hw", q=NQ, j=CPQ, p=128)

    with tile.TileContext(nc) as tc, ExitStack() as ctx:
        consts = ctx.enter_context(tc.tile_pool(name="consts", bufs=1))
        xpool = ctx.enter_context(tc.tile_pool(name="xpool", bufs=x_bufs))
        spool = ctx.enter_context(tc.tile_pool(name="spool", bufs=2))
        cpsum = ctx.enter_context(
            tc.tile_pool(name="cpsum", bufs=cams_bufs, space="PSUM"))
        mpsum = ctx.enter_context(
            tc.tile_pool(name="mpsum", bufs=mean_bufs, space="PSUM"))

        w_sb = consts.tile([128, NCHUNK, K], f32, name="w_sb")
        nc.sync.dma_start(w_sb[:], w_d.ap())
        gam_sb = consts.tile([K, 1], f32, name="gam_sb")
        nc.sync.dma_start(gam_sb[:], gam_d.ap())
        qlhs_sb = consts.tile([K, 128], f32, name="qlhs_sb")
        nc.sync.dma_start(qlhs_sb[:], qlhs_d.ap())

        for n in range(NS):
            xq = []
            for q in range(NQ):
                t = xpool.tile([128, CPQ, HW], f32, tag="xq",
                               name=f"xq_{n}_{q}")
                nc.sync.dma_start(t[:], x_src[n, q])
                xq.append(t)

            cams = cpsum.tile([K, HW], f32, tag="cams", name=f"cams_{n}")
            for j in range(NCHUNK):
                q, jj = divmod(j, CPQ)
                for c0, c1 in HALVES:
                    nc.tensor.matmul(
                        cams[:, c0:c1],
                        w_sb[:, j, :],
                        xq[q][:, jj, c0:c1],
                        start=(j == 0),
                        stop=(j == NCHUNK - 1),
                    )

            # relu on ACT (PSUM -> SBUF)
            r = spool.tile([K, HW], f32, tag="r", name=f"r_{n}")
            nc.scalar.activation(r[:], cams[:],
                                 mybir.ActivationFunctionType.Relu)
            # per-channel spatial max
            rmax = spool.tile([K, 1], f32, tag="rmax", name=f"rmax_{n}")
            nc.vector.tensor_reduce(rmax[:], r[:], axis=mybir.AxisListType.X,
                                    op=mybir.AluOpType.max)
            # thr = gama * max
            thr = spool.tile([K, 1], f32, tag="thr", name=f"thr_{n}")
            nc.vector.tensor_scalar(thr[:], rmax[:], gam_sb[:], None,
                                    op0=mybir.AluOpType.mult)
            # masked = (r <= thr) * r
            masked = spool.tile([K, HW], f32, tag="masked", name=f"masked_{n}")
            nc.vector.scalar_tensor_tensor(masked[:], r[:], thr[:], r[:],
                                           op0=mybir.AluOpType.is_le,
                                           op1=mybir.AluOpType.mult)
            # mean over k, broadcast to 128 partitions: qlhs (0.25) matmul
            meanb = mpsum.tile([128, HW], f32, tag="meanb", name=f"meanb_{n}")
            for c0, c1 in HALVES:
                nc.tensor.matmul(meanb[:, c0:c1], qlhs_sb[:],
                                 masked[:, c0:c1], start=True, stop=True)

            mb = meanb.unsqueeze(1).broadcast_to([128, CPQ, HW])
            for q in range(NQ):
                nc.vector.tensor_tensor(xq[q][:], xq[q][:], mb,
                                        op=mybir.AluOpType.mult)
                nc.sync.dma_start(out_dst[n, q], xq[q][:])

    nc.compile()
    return nc


def _get_nc():
    if "nc" not in _CACHE:
        _CACHE["nc"] = build_nc()
    return _CACHE["nc"]


def make_in_maps(x, fc_weights, gama):
    """Shard/pack full numpy inputs into per-core input maps."""
    x = np.ascontiguousarray(np.asarray(x, dtype=np.float32).reshape(NFULL, C, HW))
    fcw = np.asarray(fc_weights, dtype=np.float32).reshape(K, C)
    # w_arr[p, j, k] = fcw[k, j*128 + p]
    w_arr = np.ascontiguousarray(fcw.T.reshape(NCHUNK, 128, K).transpose(1, 0, 2))
    gam4 = np.full((K, 1), np.float32(np.asarray(gama)), dtype=np.float32)
    qlhs = np.full((K, 128), 0.25, dtype=np.float32)
    in_maps = []
    for c in range(N_CORES):
        in_maps.append({
            "x": x[c * NS:(c + 1) * NS],
            "w": w_arr,
            "gam": gam4,
            "qlhs": qlhs,
        })
    return in_maps


def kernel(x, fc_weights, gama):
    from concourse.bass_utils import run_bass_kernel_spmd

    nc = _get_nc()
    in_maps = make_in_maps(x, fc_weights, gama)
    res = run_bass_kernel_spmd(nc, in_maps, core_ids=list(range(N_CORES)))
    out = np.concatenate([r["out"] for r in res.results], axis=0)
    return out.reshape(NFULL, C, 28, 28).astype(np.float32, copy=False)


# revision 3
# speedup vs baseline: 38.1326x; 38.1326x over previous
"""Trainium2 Bass kernel for the topk-masking attention module.

Computation (per sample n):
    cams[k, hw] = relu(sum_c x[n, c, hw] * w[k, c])          # 1x1 conv, K=4
    thr[k]      = gama * max_hw(cams[k, :])
    dropped     = where(cams > thr, 0, cams)
    mean[hw]    = sum_k dropped[k, hw] / 4
    out[n,c,hw] = x[n,c,hw] * mean[hw]

Strategy: data-parallel over batch N=32 across 8 NeuronCores (4 samples
per core).  Per sample, x[n] ([4096, 784] f32, 12.85 MB) is loaded into
SBUF once as 4 quarter tiles [128, 8, 784] and kept resident: the 1x1
conv runs as 32 accumulating fp32 matmuls (contraction 4096 = 32x128 on
partitions) into PSUM [4, 784]; the per-channel max / threshold / mask
run on ACT+DVE; the channel-mean + broadcast to 128 partitions is a
single matmul with a constant [4, 128] lhsT of 0.25; and the final
elementwise multiply reuses the resident x tiles in-place before the
store.  Total HBM traffic is the 2x floor (read x once, write out once).
"""

import sys

for _p in ("/opt/trn_rl_repo",):
    if _p not in sys.path:
        sys.path.insert(0, _p)

import numpy as np

N_CORES = 8
NFULL = 32            # full batch
NS = NFULL // N_CORES  # samples per core
C = 4096
K = 4
HW = 28 * 28          # 784
NCHUNK = C // 128     # 32
NQ = 4                # quarter tiles per sample
CPQ = NCHUNK // NQ    # 8 chunks per quarter
HALVES = ((0, 512), (512, HW))  # PSUM-bank-aligned column split

_CACHE = {}


def build_nc(n_pieces=16, x_bufs=30, cams_bufs=2, mean_bufs=2,
             store_engine="scalar", gpsimd_pieces=0, mean_to_sbuf=False):
    """Trace + schedule + compile the per-core Bass program.

    n_pieces: how many SBUF tiles one sample's x is split into (must
        divide 32); x_bufs slots of [128, 32/n_pieces, 784] each.
    store_engine: which engine issues output DMAs ("sync"/"scalar"/"gpsimd")
        — separate HWDGE ring from the loads avoids FIFO coupling.
    gpsimd_pieces: how many of the per-sample multiply pieces run on
        GpSimd instead of DVE (load balancing).
    """
    from contextlib import ExitStack

    import concourse.bacc as bacc
    import concourse.tile as tile
    from concourse import mybir

    f32 = mybir.dt.float32
    nc = bacc.Bacc("TRN2", target_bir_lowering=False, debug=False,
                   num_devices=N_CORES)

    NP = n_pieces
    CPP = NCHUNK // NP  # chunks per piece

    x_d = nc.dram_tensor("x", [NS, C, HW], f32, kind="ExternalInput")
    w_d = nc.dram_tensor("w", [128, NCHUNK, K], f32, kind="ExternalInput")
    gam_d = nc.dram_tensor("gam", [K, 1], f32, kind="ExternalInput")
    qlhs_d = nc.dram_tensor("qlhs", [K, 128], f32, kind="ExternalInput")
    out_d = nc.dram_tensor("out", [NS, C, HW], f32, kind="ExternalOutput")

    # [NS, C, HW] viewed as [NS, NP, 128(part), CPP, HW]
    x_src = x_d.ap().rearrange("n (q j p) hw -> n q p j hw", q=NP, j=CPP, p=128)
    out_dst = out_d.ap().rearrange("n (q j p) hw -> n q p j hw", q=NP, j=CPP, p=128)

    store_eng = getattr(nc, store_engine)

    with tile.TileContext(nc) as tc, ExitStack() as ctx:
        consts = ctx.enter_context(tc.tile_pool(name="consts", bufs=1))
        xpool = ctx.enter_context(tc.tile_pool(name="xpool", bufs=x_bufs))
        spool = ctx.enter_context(tc.tile_pool(name="spool", bufs=2))
        cpsum = ctx.enter_context(
            tc.tile_pool(name="cpsum", bufs=cams_bufs, space="PSUM"))
        mpsum = ctx.enter_context(
            tc.tile_pool(name="mpsum", bufs=mean_bufs, space="PSUM"))

        w_sb = consts.tile([128, NCHUNK, K], f32, name="w_sb")
        nc.sync.dma_start(w_sb[:], w_d.ap())
        gam_sb = consts.tile([K, 1], f32, name="gam_sb")
        nc.sync.dma_start(gam_sb[:], gam_d.ap())
        qlhs_sb = consts.tile([K, 128], f32, name="qlhs_sb")
        nc.sync.dma_start(qlhs_sb[:], qlhs_d.ap())

        for n in range(NS):
            xq = []
            for q in range(NP):
                t = xpool.tile([128, CPP, HW], f32, tag="xq",
                               name=f"xq_{n}_{q}")
                nc.sync.dma_start(t[:], x_src[n, q])
                xq.append(t)

            cams = cpsum.tile([K, HW], f32, tag="cams", name=f"cams_{n}")
            for j in range(NCHUNK):
                q, jj = divmod(j, CPP)
                for c0, c1 in HALVES:
                    nc.tensor.matmul(
                        cams[:, c0:c1],
                        w_sb[:, j, :],
                        xq[q][:, jj, c0:c1],
                        start=(j == 0),
                        stop=(j == NCHUNK - 1),
                    )

            # relu on ACT (PSUM -> SBUF)
            r = spool.tile([K, HW], f32, tag="r", name=f"r_{n}")
            nc.scalar.activation(r[:], cams[:],
                                 mybir.ActivationFunctionType.Relu)
            # per-channel spatial max
            rmax = spool.tile([K, 1], f32, tag="rmax", name=f"rmax_{n}")
            nc.vector.tensor_reduce(rmax[:], r[:], axis=mybir.AxisListType.X,
                                    op=mybir.AluOpType.max)
            # thr = gama * max
            thr = spool.tile([K, 1], f32, tag="thr", name=f"thr_{n}")
            nc.vector.tensor_scalar(thr[:], rmax[:], gam_sb[:], None,
                                    op0=mybir.AluOpType.mult)
            # masked = (r <= thr) * r
            masked = spool.tile([K, HW], f32, tag="masked", name=f"masked_{n}")
            nc.vector.scalar_tensor_tensor(masked[:], r[:], thr[:], r[:],
                                           op0=mybir.AluOpType.is_le,
                                           op1=mybir.AluOpType.mult)
            # mean over k, broadcast to 128 partitions: qlhs (0.25) matmul
            meanb = mpsum.tile([128, HW], f32, tag="meanb", name=f"meanb_{n}")
            for c0, c1 in HALVES:
                nc.tensor.matmul(meanb[:, c0:c1], qlhs_sb[:],
                                 masked[:, c0:c1], start=True, stop=True)

            mean_src = meanb
            if mean_to_sbuf:
                mean_sb = spool.tile([128, HW], f32, tag="mean_sb",
                                     name=f"mean_sb_{n}")
                nc.vector.tensor_copy(mean_sb[:], meanb[:])
                mean_src = mean_sb

            mb = mean_src.unsqueeze(1).broadcast_to([128, CPP, HW])
            for q in range(NP):
                eng = nc.gpsimd if q < gpsimd_pieces else nc.vector
                eng.tensor_tensor(xq[q][:], xq[q][:], mb,
                                  op=mybir.AluOpType.mult)
                store_eng.dma_start(out_dst[n, q], xq[q][:])

    nc.compile()
    return nc


def _get_nc():
    if "nc" not in _CACHE:
        _CACHE["nc"] = build_nc()
    return _CACHE["nc"]


def make_in_maps(x, fc_weights, gama):
    """Shard/pack full numpy inputs into per-core input maps."""
    x = np.ascontiguousarray(np.asarray(x, dtype=np.float32).reshape(NFULL, C, HW))
    fcw = np.asarray(fc_weights, dtype=np.float32).reshape(K, C)
    # w_arr[p, j, k] = fcw[k, j*128 + p]
    w_arr = np.ascontiguousarray(fcw.T.reshape(NCHUNK, 128, K).transpose(1, 0, 2))
    gam4 = np.full((K, 1), np.float32(np.asarray(gama)), dtype=np.float32)
    qlhs = np.full((K, 128), 0.25, dtype=np.float32)
    in_maps = []
    for c in range(N_CORES):
        in_maps.append({
            "x": x[c * NS:(c + 1) * NS],
            "w": w_arr,
            "gam": gam4,
            "qlhs": qlhs,
        })
    return in_maps


def kernel(x, fc_weights, gama):
    from concourse.bass_utils import run_bass_kernel_spmd

    nc = _get_nc()
    in_maps = make_in_maps(x, fc_weights, gama)
    res = run_bass_kernel_spmd(nc, in_maps, core_ids=list(range(N_CORES)))
    out = np.concatenate([r["out"] for r in res.results], axis=0)
    return out.reshape(NFULL, C, 28, 28).astype(np.float32, copy=False)
